# revision 33
# baseline (speedup 1.0000x reference)
"""Multi-head attention (B=4, S=2048, D=1024, H=16) on 8 TRN2 NeuronCores.

Sharding: 2D grid (batch x head-group). Core c = g*4 + b handles batch b and
head group g (8 heads = 512 of the 1024 embedding columns, as 4 pairs of 2).

Design (from HW microbenchmarks + traces):
- PE matmul cost = moving-dim rows only; contraction depth is free. Mixing
  64-row-tiled and 128x128 matmuls costs ~400ns per array mode alternation,
  so EVERY matmul uses the full 128x128 configuration: per-head score
  matmuls zero-pad K to 128 contraction rows (kt0 = [K_h0;0], kt1 =
  [0;K_h1]) and share the full 128-row Q^T as the moving operand.
- ScalarE runs ONLY the exp (the rate limiter: 64 x [128,1024] exps per
  pair at ~1.09us each). All PSUM evictions are on DVE.
- AV uses exp(S) as the STATIONARY operand: attn[q,dh] = es^T @ [V|1],
  32 x 65-row matmuls per q-tile accumulating [128,130] =
  [attn_h0|sum_h0|attn_h1|sum_h1]. Only the first matmul into a PSUM bank
  may carry start=True (start clears has-written for the whole bank).
  The sumexp lands as a per-partition column, so normalization is a plain
  tensor_scalar multiply straight from PSUM; the normalized q-tile (bf16)
  is PE-transposed (1 cyc/row in bf16, ~58ns) into the attnT layout used
  by the output projection.  DMA XBAR transposes measured 1.25us each on
  the Sync engine — far worse than their 14ns/tile cost model — so all
  transposes stay on the PE.
- Pipeline shape (PE busy ~334us > ACT busy ~277us, so PE is the
  resource to protect):
  * head: 30 x N=128 warmup MMs hold PE activity while the first DMAs
    land (the x nt0 chunks are spread across all three DMA queues;
    biases first on scalar); attention(pair0) starts as soon as
    q-nt0/k-nt0 are projected, the rest of pair0 QKV runs as filler
    between score/exp slots (2 filler pops per kt while no AV work
    exists).
  * steady state: per k-tile slot = score pair + exp + 1 AV slot + 1
    filler (next pair's QKV projections, V transposes, output-projection
    half-closures of <=2 matmuls each so one pop never exceeds the
    ~1.2us exp cadence).
  * tail: the last (pair,qc) consumes its own AV slots eagerly as each
    4-exp group lands (qt4 0/1 interleave on the two AV PSUM banks), and
    output-projection closures for a q-tile are queued as soon as that
    q-tile's attnT lands.
- HW pitfalls encoded here: single-partition DVE reads of PSUM at
  partition base 64 return garbage (keep all PSUM reads at base 0);
  partition-broadcast DMAs must go to sync/scalar queues; f32r memset is
  rejected by the ISA (broadcast-copy zeros instead).
- dtypes: x/W/es/V/attnT/Wo bf16; Q^T/K^T f32r; PSUM fp32; out bf16
  (summed in f32 on host).
"""
import numpy as np

B, S, D, H, DH = 4, 2048, 1024, 16, 64
NCORES = 8
GCOLS = D // 2          # 512 cols per head-group core
NPAIRS = GCOLS // 128   # 4 head-pairs per core
NKT = S // 128          # 16 k-tiles
NQC = S // 512          # 4 q-chunks of 512
DC = D // 128           # 8 contraction chunks for projections

_COMPILED = None


def _build():
    import concourse.bass as bass
    import concourse.bacc as bacc
    import concourse.tile as tile
    from concourse import mybir
    from concourse.masks import make_identity
    from contextlib import ExitStack
    from collections import deque

    F32 = mybir.dt.float32
    F32R = mybir.dt.float32r
    BF16 = mybir.dt.bfloat16
    EXP = mybir.ActivationFunctionType.Exp

    nc = bacc.Bacc("TRN2", target_bir_lowering=False, debug=False)
    xT = nc.dram_tensor("xT", [D, S], BF16, kind="ExternalInput").ap()
    # host pre-arranges qkv weights as [pair, partition, dc*128] so the
    # per-pair weight DMA is 128 contiguous 2KB lines (a [D, GCOLS] layout
    # needs 1024 strided 256B descriptors and takes ~20us)
    wq = nc.dram_tensor("wq", [NPAIRS, 128, DC * 128], BF16,
                        kind="ExternalInput").ap()
    wk = nc.dram_tensor("wk", [NPAIRS, 128, DC * 128], BF16,
                        kind="ExternalInput").ap()
    wv = nc.dram_tensor("wv", [NPAIRS, 128, DC * 128], BF16,
                        kind="ExternalInput").ap()
    wo = nc.dram_tensor("wo", [GCOLS, D], BF16, kind="ExternalInput").ap()
    bq = nc.dram_tensor("bq", [GCOLS], F32, kind="ExternalInput").ap()
    bk = nc.dram_tensor("bk", [GCOLS], F32, kind="ExternalInput").ap()
    bv = nc.dram_tensor("bv", [GCOLS], F32, kind="ExternalInput").ap()
    out = nc.dram_tensor("out", [S, D], BF16,
                         kind="ExternalOutput").ap()

    with tile.TileContext(nc) as tc, ExitStack() as ctx:
        const = ctx.enter_context(tc.tile_pool(name="const", bufs=1))
        persist = ctx.enter_context(tc.tile_pool(name="persist", bufs=1))
        wpool = ctx.enter_context(tc.tile_pool(name="wpool", bufs=1))
        qkv = ctx.enter_context(tc.tile_pool(name="qkv", bufs=2))
        vpool = ctx.enter_context(tc.tile_pool(name="vpool", bufs=2))
        espool = ctx.enter_context(tc.tile_pool(name="espool", bufs=33))
        nrm = ctx.enter_context(tc.tile_pool(name="nrm", bufs=2))
        osb = ctx.enter_context(tc.tile_pool(name="osb", bufs=2))
        scps = ctx.enter_context(tc.tile_pool(name="scps", bufs=2,
                                              space="PSUM"))
        avps = ctx.enter_context(tc.tile_pool(name="avps", bufs=1,
                                              space="PSUM"))
        fips = ctx.enter_context(tc.tile_pool(name="fips", bufs=2,
                                              space="PSUM"))

        idf = const.tile([128, 128], F32)
        make_identity(nc, idf)
        idb = const.tile([128, 128], BF16)
        make_identity(nc, idb)
        bq_sb = const.tile([128, NPAIRS], F32)
        bk_sb = const.tile([128, NPAIRS], F32)
        bv_sb = const.tile([128, NPAIRS], F32)
        zb = const.tile([128, 512], BF16)
        nc.vector.memset(zb, 0.0)
        zf = const.tile([128, 512], F32)
        nc.vector.memset(zf, 0.0)

        def emit_bias_dmas():
            # first on scalar — the q-nt0 sub3 bias-add is on the
            # critical path to the first score block; partition-
            # restructuring DMAs must stay on sync/scalar queues
            nc.scalar.dma_start(out=bq_sb,
                                in_=bq.rearrange("(p r) -> r p", r=128))
            nc.scalar.dma_start(out=bk_sb,
                                in_=bk.rearrange("(p r) -> r p", r=128))
            nc.scalar.dma_start(out=bv_sb,
                                in_=bv.rearrange("(p r) -> r p", r=128))

        # x^T resident in bf16 (host pre-casts; DMA direct).  Emitted via
        # emit_x_dmas() after pair-0's weight DMAs so projections can
        # start as soon as the first column chunks land.
        xT_sb = persist.tile([128, DC, S], BF16)
        xT_dram = xT.rearrange("(dc p) n -> p dc n", p=128)

        def emit_x_dmas():
            # a queue processes one kickoff's descriptor set at a time at
            # ~90GB/s, so the nt0 chunks the prime needs first are spread
            # over ALL THREE queues as dc-pair kickoffs; later nt groups
            # ride sync/scalar (gpsimd then takes wv+wo).
            for eng, dsl in ((nc.sync, slice(0, 2)),
                             (nc.scalar, slice(4, 6)),
                             (nc.gpsimd, slice(2, 4)),
                             (nc.gpsimd, slice(6, 8))):
                eng.dma_start(out=xT_sb[:, dsl, 0:512],
                              in_=xT_dram[:, dsl, 0:512])
            qeng2 = [nc.sync, nc.scalar]
            for h in range(2):
                dsl = slice(h * 4, (h + 1) * 4)
                qeng2[h].dma_start(
                    out=xT_sb[:, dsl, 512:1024],
                    in_=xT_dram[:, dsl, 512:1024])
            for h in range(2):
                dsl = slice(h * 4, (h + 1) * 4)
                qeng2[h].dma_start(
                    out=xT_sb[:, dsl, 1024:S], in_=xT_dram[:, dsl, 1024:S])

        wo_sb = persist.tile([128, NPAIRS, D], BF16)

        def emit_wo_dma():
            # 1MB, 512 descriptors — needed only when the output
            # projection starts (pair 3), so it must NOT clog the gpsimd
            # queue ahead of wv (emitted after pair0's setup)
            nc.gpsimd.dma_start(out=wo_sb,
                                in_=wo.rearrange("(p r) n -> r p n", r=128))

        attnT = [persist.tile([128, S], BF16, name=f"attnT{p}",
                              tag=f"attnT{p}") for p in range(NPAIRS)]

        # warmup to hold PE activity while the first DMAs land.  N=128
        # keeps the queue fine-grained so real matmuls start the moment
        # their DMA deps resolve (~9us); sized to end right about then —
        # the real projection stream then carries HAM past its warm
        # threshold.
        warm_ps = scps.tile([128, 1024], F32, name="warm_ps", tag="sc")
        for _ in range(30):
            nc.tensor.matmul(warm_ps[:, 0:128], zb[:, 0:128],
                             zb[:, 0:128],
                             start=True, stop=True, skip_group_check=True)

        pair_tiles = {}
        wre = {"q": wq, "k": wk, "v": wv}
        bias_sb = {"q": bq_sb, "k": bk_sb, "v": bv_sb}

        def gen_pair_work(p, prime=False):
            """Closures producing Q^T (f32r), zero-padded per-head K^T
            (f32r), and V (f32, k-major with ones cols) for pair p.
            Emitted as PE filler during pair p-1's attention.

            For prime (pair 0): returns (inline_ops, filler_ops) where
            inline_ops = setup + q-nt0 + k-nt0 + done (the minimum for
            the first score block) and filler_ops carries the rest in
            dependency order: remaining K chunks first (scores consume
            them at 4 k-tiles per nt chunk), then q-nt1 (needed at qc1),
            then all V work (needed when qc0's AV runs during qc1),
            then q-nt2/q-nt3."""
            ops = []
            st = {}

            def setup():
                weng = {"q": nc.sync, "k": nc.scalar, "v": nc.gpsimd}
                for nm in ("q", "k", "v"):
                    t = wpool.tile([128, DC, 128], BF16, name=f"w{nm}",
                                   tag=f"w{nm}")
                    if not (prime and nm == "v"):
                        weng[nm].dma_start(
                            out=t.rearrange("p a b -> p (a b)"),
                            in_=wre[nm][p])
                    st[nm] = t

                st["qt"] = qkv.tile([128, S], F32R, name="qt_sb", tag="qt")
                st["kt0"] = qkv.tile([128, S], F32R, name="kt0_sb",
                                     tag="kt0")
                st["kt1"] = qkv.tile([128, S], F32R, name="kt1_sb",
                                     tag="kt1")
                st["vt"] = qkv.tile([128, S], BF16, name="vt_sb", tag="vt",
                                    bufs=1)
                st["vsb"] = vpool.tile([128, NKT, 130], BF16, name="v_sb",
                                       tag="vsb")
                nc.vector.memset(st["vsb"][:, :, 64:65], 1.0)
                nc.vector.memset(st["vsb"][:, :, 129:130], 1.0)
                zrep = bass.AP(tensor=zf.tensor, offset=zf.offset,
                               ap=[zf.ap[0], [0, 4], [1, 512]])
                nc.vector.tensor_copy(
                    st["kt0"].rearrange("p (a b) -> p a b", b=512), zrep)
                nc.vector.tensor_copy(
                    st["kt1"].rearrange("p (a b) -> p a b", b=512), zrep)
            ops.append(setup)

            def wv_kick():
                # prime only: wv rides gpsimd AFTER the nt0 x chunks
                # (needed ~20us later than them)
                nc.gpsimd.dma_start(
                    out=st["v"].rearrange("p a b -> p (a b)"),
                    in_=wre["v"][p])

            pstate = {}

            def proj(nm, nt, sub):
                def go():
                    if sub == 0:
                        pstate["pp"] = fips.tile([128, 512], F32, name="pp",
                                                 tag="pp")
                    pp = pstate["pp"]
                    for dc in range(2 * sub, 2 * sub + 2):
                        nc.tensor.matmul(
                            pp, st[nm][:, dc, :],
                            xT_sb[:, dc, nt * 512:(nt + 1) * 512],
                            start=(dc == 0), stop=(dc == DC - 1))
                    if sub == 3:
                        csl = slice(nt * 512, (nt + 1) * 512)
                        b = bias_sb[nm][:, p:p + 1]
                        if nm == "q":
                            nc.vector.tensor_scalar_add(st["qt"][:, csl],
                                                        pp, b)
                        elif nm == "v":
                            nc.vector.tensor_scalar_add(st["vt"][:, csl],
                                                        pp, b)
                        else:
                            nc.vector.tensor_scalar_add(
                                st["kt0"][0:64, csl], pp[0:64, :],
                                b[0:64, :])
                            nc.vector.tensor_scalar_add(
                                st["kt1"][64:128, csl], pp[64:128, :],
                                b[64:128, :])
                return go

            def vtr(g4):
                # PE transpose (bf16 vt -> 1 cyc/row, half the old f32
                # cost; DMA XBAR transposes measured 1.25us each on the
                # Sync engine and choke the v_sb dependency chain)
                def go():
                    tr = fips.tile([128, 512], BF16, name="tr", tag="pp")
                    for j in range(4):
                        kb = g4 * 4 + j
                        nc.tensor.matmul(
                            tr[:, j * 128:(j + 1) * 128],
                            st["vt"][:, kb * 128:(kb + 1) * 128], idb,
                            is_transpose=True, start=True, stop=True)
                    src0 = bass.AP(tensor=tr.tensor, offset=tr.offset,
                                   ap=[tr.ap[0], [128, 4], [1, 64]])
                    src1 = bass.AP(tensor=tr.tensor,
                                   offset=tr.offset + 64,
                                   ap=[tr.ap[0], [128, 4], [1, 64]])
                    vs = st["vsb"]
                    nc.vector.tensor_copy(
                        vs[:, g4 * 4:g4 * 4 + 4, 0:64], src0)
                    nc.vector.tensor_copy(
                        vs[:, g4 * 4:g4 * 4 + 4, 65:129], src1)
                return go

            def done():
                pair_tiles[p] = st

            if prime:
                # inline: the minimum to open the first score block
                for nm in ("q", "k"):
                    for sub in range(4):
                        ops.append(proj(nm, 0, sub))
                ops.append(done)
                # filler order tuned against consumption pops: remaining
                # K chunks (scores eat 4 k-tiles per nt), v-nt0+vtr0
                # (qc0's AV starts at qc1-kt0), q-nt1 (qc1's scores),
                # the rest of V (AV eats vtr g4 at qc1-kt 4*g4), then
                # q-nt2/3 (qc2/qc3)
                rest = []
                for nt in range(1, 4):
                    for sub in range(4):
                        rest.append(proj("k", nt, sub))
                for sub in range(4):
                    rest.append(proj("v", 0, sub))
                rest.append(vtr(0))
                for sub in range(4):
                    rest.append(proj("q", 1, sub))
                for nt in range(1, 4):
                    for sub in range(4):
                        rest.append(proj("v", nt, sub))
                    rest.append(vtr(nt))
                for nt in (2, 3):
                    for sub in range(4):
                        rest.append(proj("q", nt, sub))
                return ops, rest, wv_kick

            # non-prime: Q/K first, then done (so the handles exist well
            # before the next pair's emission and the boundary drain is a
            # no-op), then V work — consumed only by the pair's own AV
            # slots, which lag a full q-chunk behind.
            for nm in ("q", "k"):
                for nt in range(4):
                    for sub in range(4):
                        ops.append(proj(nm, nt, sub))
            ops.append(done)
            for nt in range(4):
                for sub in range(4):
                    ops.append(proj("v", nt, sub))
                ops.append(vtr(nt))
            return ops

        filler = deque()
        avq = deque()

        def gen_outproj_qci(qci):
            """4 half-closures (2 matmuls each) computing the output
            projection for q rows qci*128..+128.  Requires attnT[p][:,
            those cols] for all pairs.  Split so a single filler pop
            never exceeds the exp cadence."""
            ops = []
            ostate = {}
            for nt in range(2):
                def goA(nt=nt):
                    op = fips.tile([128, 512], F32, name="op", tag="pp")
                    ostate[nt] = op
                    for pi in (0, 1):
                        nc.tensor.matmul(
                            op,
                            attnT[pi][:, qci * 128:(qci + 1) * 128],
                            wo_sb[:, pi, nt * 512:(nt + 1) * 512],
                            start=(pi == 0), stop=False,
                            skip_group_check=True)

                def goB(nt=nt):
                    op = ostate[nt]
                    for pi in (2, 3):
                        nc.tensor.matmul(
                            op,
                            attnT[pi][:, qci * 128:(qci + 1) * 128],
                            wo_sb[:, pi, nt * 512:(nt + 1) * 512],
                            start=False, stop=(pi == NPAIRS - 1),
                            skip_group_check=True)
                    o = osb.tile([128, 512], BF16, name="o", tag="o")
                    # the last q-chunk's evictions + out-DMAs run in the
                    # drain when the ACT engine is done with exps — use
                    # it (and its queue) to unload DVE/gpsimd tail
                    # serialization
                    if qci >= 12:
                        nc.scalar.copy(o, op)
                        nc.scalar.dma_start(
                            out=out[qci * 128:(qci + 1) * 128,
                                    nt * 512:(nt + 1) * 512], in_=o)
                    else:
                        nc.vector.tensor_copy(o, op)
                        nc.gpsimd.dma_start(
                            out=out[qci * 128:(qci + 1) * 128,
                                    nt * 512:(nt + 1) * 512], in_=o)
                ops.append(goA)
                ops.append(goB)
            return ops

        def gen_av_slot_factory(p, qc, es_list, v_sb):
            """Returns slot(qt4, sub) closure factory for q-chunk qc of
            pair p.  attn[q,dh] with es as the STATIONARY operand: per
            (qt4, sub) slot, 8 small matmuls (4 k-tiles x 2 heads, 65
            moving rows each) accumulate [128,130] =
            [attn_h0|sum_h0|attn_h1|sum_h1].  The sumexp lands as a
            per-partition column -> normalize is a plain tensor_scalar
            multiply straight from PSUM, and a DMA XBAR transpose drops
            the bf16 result into the attnT layout."""
            q0 = qc * 512
            state = {}

            def slot(qt4, sub):
                def go():
                    if sub == 0:
                        avfull = avps.tile([128, 512], F32, name="av",
                                           tag=f"av{qt4 % 2}")
                        state[qt4] = avfull[:, 0:130]
                    av = state[qt4]
                    for kt in range(4 * sub, 4 * sub + 4):
                        for hh in range(2):
                            esl = es_list[kt][:, hh * 512 + qt4 * 128:
                                              hh * 512 + (qt4 + 1) * 128]
                            nc.tensor.matmul(
                                av[:, hh * 65:(hh + 1) * 65], esl,
                                v_sb[:, kt, hh * 65:(hh + 1) * 65],
                                start=(kt == 0 and hh == 0),
                                stop=(kt == NKT - 1),
                                skip_group_check=True)
                    if sub == 3:
                        rec = nrm.tile([128, 2], F32, name="rec",
                                       tag="rec", bufs=3)
                        av_sums = bass.AP(tensor=av.tensor,
                                          offset=av.offset + 64,
                                          ap=[av.ap[0], [65, 2]])
                        nc.vector.reciprocal(rec, av_sums)
                        att = nrm.tile([128, 128], BF16, name="att",
                                       tag="att", bufs=3)
                        nc.vector.tensor_scalar_mul(
                            att[:, 0:64], av[:, 0:64], rec[:, 0:1])
                        nc.vector.tensor_scalar_mul(
                            att[:, 64:128], av[:, 65:129], rec[:, 1:2])
                        # PE transpose (bf16: 1 cyc/row, ~58ns; the DMA
                        # XBAR path costs 1.25us on the Sync engine and
                        # serializes the drain)
                        trn = fips.tile([128, 128], BF16, name="trn",
                                        tag="pp")
                        nc.tensor.matmul(trn, att, idb,
                                         is_transpose=True,
                                         start=True, stop=True)
                        nc.vector.tensor_copy(
                            attnT[p][:,
                                     q0 + qt4 * 128:q0 + (qt4 + 1) * 128],
                            trn)
                        if p == NPAIRS - 1:
                            filler.extend(gen_outproj_qci(qc * 4 + qt4))
                return go
            return slot

        def attention(p):
            while p not in pair_tiles:
                filler.popleft()()
            st = pair_tiles[p]
            qt_sb, kt0, kt1 = st["qt"], st["kt0"], st["kt1"]
            v_sb = st["vsb"]
            if p < NPAIRS - 1:
                filler.extend(gen_pair_work(p + 1))
            for qc in range(NQC):
                last = (p == NPAIRS - 1 and qc == NQC - 1)
                q0 = qc * 512
                es_q = []
                mkslot = gen_av_slot_factory(p, qc, es_q, v_sb)
                for kt in range(NKT):
                    sc = scps.tile([128, 1024], F32, name="sc", tag="sc")
                    nc.tensor.matmul(
                        sc[:, 0:512],
                        kt0[:, kt * 128:(kt + 1) * 128],
                        qt_sb[:, q0:q0 + 512], start=True, stop=True)
                    nc.tensor.matmul(
                        sc[:, 512:1024],
                        kt1[:, kt * 128:(kt + 1) * 128],
                        qt_sb[:, q0:q0 + 512], start=True, stop=True)
                    es = espool.tile([128, 1024], BF16, name="es",
                                     tag="es")
                    nc.scalar.activation(es, sc, EXP, scale=0.125)
                    es_q.append(es)
                    # adaptive: drain the filler backlog (pair-boundary
                    # bursts starve the exp stream otherwise)
                    npop = 1 + (1 if not avq else 0) \
                        + (1 if len(filler) > 28 else 0)
                    for _ in range(npop):
                        if filler:
                            filler.popleft()()
                    for _ in range(2 if last else 1):
                        if avq:
                            avq.popleft()()
                    if last and kt % 4 == 3:
                        g = kt // 4
                        avq.append(mkslot(0, g))
                        avq.append(mkslot(1, g))
                        if kt == NKT - 1:
                            for t in (2, 3):
                                for g2 in range(4):
                                    avq.append(mkslot(t, g2))
                if not last:
                    for qt4 in range(4):
                        for sub in range(4):
                            avq.append(mkslot(qt4, sub))

        # prime pair 0 (runs while x DMAs land; warmup holds the clock).
        # Pair-0 weight DMAs go out before the bulk x load so the first
        # projection matmuls are not starved.
        prime_ops, prime_rest, wv_kick = gen_pair_work(0, prime=True)
        emit_bias_dmas()
        prime_ops[0]()
        emit_x_dmas()
        wv_kick()
        emit_wo_dma()
        for op in prime_ops[1:]:
            op()
        filler.extend(prime_rest)

        for p in range(NPAIRS):
            attention(p)
        while avq:
            avq.popleft()()
            if filler:
                filler.popleft()()
        while filler:
            filler.popleft()()

    nc.compile()
    return nc


def _get_compiled():
    global _COMPILED
    if _COMPILED is None:
        _COMPILED = _build()
    return _COMPILED


def make_in_maps(**inputs):
    import ml_dtypes
    bf = ml_dtypes.bfloat16
    x = np.asarray(inputs["inputs"], np.float32)
    xTb = [np.ascontiguousarray(x[b].T).astype(bf) for b in range(B)]
    gslice = {}
    for nm in ("Wq", "Wk", "Wv", "Wo", "bq", "bk", "bv"):
        a = np.asarray(inputs[nm], np.float32)
        for g in range(2):
            sl = slice(g * GCOLS, (g + 1) * GCOLS)
            if nm == "Wo":
                gslice[(nm, g)] = np.ascontiguousarray(a[sl, :]).astype(bf)
            elif nm.startswith("W"):
                w = a[:, sl].reshape(8, 128, 4, 128).transpose(1, 2, 0, 3)
                gslice[(nm, g)] = np.ascontiguousarray(
                    w.transpose(1, 0, 2, 3).reshape(4, 128, 1024)).astype(bf)
            else:
                gslice[(nm, g)] = np.ascontiguousarray(a[sl])
    in_maps = []
    for c in range(NCORES):
        g, b = c // B, c % B
        in_maps.append({
            "xT": xTb[b],
            "wq": gslice[("Wq", g)], "wk": gslice[("Wk", g)],
            "wv": gslice[("Wv", g)], "wo": gslice[("Wo", g)],
            "bq": gslice[("bq", g)], "bk": gslice[("bk", g)],
            "bv": gslice[("bv", g)],
        })
    return in_maps


def combine(results, bo):
    out = np.empty((B, S, D), np.float32)
    bo = np.asarray(bo, np.float32)
    for b in range(B):
        out[b] = (np.asarray(results[b]["out"], np.float32)
                  + np.asarray(results[B + b]["out"], np.float32) + bo)
    return out


def kernel(**inputs):
    from concourse import bass_utils
    nc = _get_compiled()
    in_maps = make_in_maps(**inputs)
    res = bass_utils.run_bass_kernel_spmd(
        nc, in_maps, core_ids=list(range(NCORES)))
    return combine(res.results, inputs["bo"])


# revision 37
# speedup vs baseline: 1.0165x; 1.0165x over previous
"""Multi-head attention (B=4, S=2048, D=1024, H=16) on 8 TRN2 NeuronCores.

Sharding: 2D grid (batch x head-group). Core c = g*4 + b handles batch b and
head group g (8 heads = 512 of the 1024 embedding columns, as 4 pairs of 2).

Design (from HW microbenchmarks + traces):
- PE matmul cost = moving-dim rows only; contraction depth is free. Mixing
  64-row-tiled and 128x128 matmuls costs ~400ns per array mode alternation,
  so EVERY matmul uses the full 128x128 configuration: per-head score
  matmuls zero-pad K to 128 contraction rows (kt0 = [K_h0;0], kt1 =
  [0;K_h1]) and share the full 128-row Q^T as the moving operand.
- ScalarE runs ONLY the exp (the rate limiter: 64 x [128,1024] exps per
  pair at ~1.09us each). All PSUM evictions are on DVE.
- AV uses exp(S) as the STATIONARY operand: attn[q,dh] = es^T @ [V|1],
  32 x 65-row matmuls per q-tile accumulating [128,130] =
  [attn_h0|sum_h0|attn_h1|sum_h1]. Only the first matmul into a PSUM bank
  may carry start=True (start clears has-written for the whole bank).
  The sumexp lands as a per-partition column, so normalization is a plain
  tensor_scalar multiply straight from PSUM; the normalized q-tile (bf16)
  is PE-transposed (1 cyc/row in bf16, ~58ns) into the attnT layout used
  by the output projection.  DMA XBAR transposes measured 1.25us each on
  the Sync engine — far worse than their 14ns/tile cost model — so all
  transposes stay on the PE.
- Pipeline shape (PE busy ~334us > ACT busy ~277us, so PE is the
  resource to protect):
  * head: 30 x N=128 warmup MMs hold PE activity while the first DMAs
    land (the x nt0 chunks are spread across all three DMA queues;
    biases first on scalar); attention(pair0) starts as soon as
    q-nt0/k-nt0 are projected, the rest of pair0 QKV runs as filler
    between score/exp slots (2 filler pops per kt while no AV work
    exists).
  * steady state: per k-tile slot = score pair + exp + 1 AV slot + 1
    filler (next pair's QKV projections, V transposes, output-projection
    half-closures of <=2 matmuls each so one pop never exceeds the
    ~1.2us exp cadence).
  * tail: the last (pair,qc) consumes its own AV slots eagerly as each
    4-exp group lands (qt4 0/1 interleave on the two AV PSUM banks), and
    output-projection closures for a q-tile are queued as soon as that
    q-tile's attnT lands.
- HW pitfalls encoded here: single-partition DVE reads of PSUM at
  partition base 64 return garbage (keep all PSUM reads at base 0);
  partition-broadcast DMAs must go to sync/scalar queues; f32r memset is
  rejected by the ISA (broadcast-copy zeros instead).
- dtypes: x/W/es/V/attnT/Wo bf16; Q^T/K^T f32r; PSUM fp32; out bf16
  (summed in f32 on host).
"""
import numpy as np

B, S, D, H, DH = 4, 2048, 1024, 16, 64
NCORES = 8
GCOLS = D // 2          # 512 cols per head-group core
NPAIRS = GCOLS // 128   # 4 head-pairs per core
NKT = S // 128          # 16 k-tiles
NQC = S // 512          # 4 q-chunks of 512
DC = D // 128           # 8 contraction chunks for projections

_COMPILED = None


def _build():
    import concourse.bass as bass
    import concourse.bacc as bacc
    import concourse.tile as tile
    from concourse import mybir
    from concourse.masks import make_identity
    from contextlib import ExitStack
    from collections import deque

    F32 = mybir.dt.float32
    F32R = mybir.dt.float32r
    BF16 = mybir.dt.bfloat16
    EXP = mybir.ActivationFunctionType.Exp

    nc = bacc.Bacc("TRN2", target_bir_lowering=False, debug=False)
    # host pre-arranges x^T as [nt, p, dc*512] so one DMA kickoff per nt
    # chunk moves 1MB in 128 contiguous 8KB descriptors (the [D, S]
    # layout needs 1024 1KB descriptors per chunk and lands ~10us late)
    xT = nc.dram_tensor("xT", [4, 128, DC * 512], BF16,
                        kind="ExternalInput").ap()
    # host pre-arranges qkv weights as [pair, partition, dc*128] so the
    # per-pair weight DMA is 128 contiguous 2KB lines (a [D, GCOLS] layout
    # needs 1024 strided 256B descriptors and takes ~20us)
    wq = nc.dram_tensor("wq", [NPAIRS, 128, DC * 128], BF16,
                        kind="ExternalInput").ap()
    wk = nc.dram_tensor("wk", [NPAIRS, 128, DC * 128], BF16,
                        kind="ExternalInput").ap()
    wv = nc.dram_tensor("wv", [NPAIRS, 128, DC * 128], BF16,
                        kind="ExternalInput").ap()
    wo = nc.dram_tensor("wo", [GCOLS, D], BF16, kind="ExternalInput").ap()
    bq = nc.dram_tensor("bq", [GCOLS], F32, kind="ExternalInput").ap()
    bk = nc.dram_tensor("bk", [GCOLS], F32, kind="ExternalInput").ap()
    bv = nc.dram_tensor("bv", [GCOLS], F32, kind="ExternalInput").ap()
    out = nc.dram_tensor("out", [S, D], BF16,
                         kind="ExternalOutput").ap()

    with tile.TileContext(nc) as tc, ExitStack() as ctx:
        const = ctx.enter_context(tc.tile_pool(name="const", bufs=1))
        persist = ctx.enter_context(tc.tile_pool(name="persist", bufs=1))
        wpool = ctx.enter_context(tc.tile_pool(name="wpool", bufs=1))
        qkv = ctx.enter_context(tc.tile_pool(name="qkv", bufs=2))
        vpool = ctx.enter_context(tc.tile_pool(name="vpool", bufs=2))
        espool = ctx.enter_context(tc.tile_pool(name="espool", bufs=33))
        nrm = ctx.enter_context(tc.tile_pool(name="nrm", bufs=2))
        osb = ctx.enter_context(tc.tile_pool(name="osb", bufs=2))
        scps = ctx.enter_context(tc.tile_pool(name="scps", bufs=2,
                                              space="PSUM"))
        avps = ctx.enter_context(tc.tile_pool(name="avps", bufs=1,
                                              space="PSUM"))
        fips = ctx.enter_context(tc.tile_pool(name="fips", bufs=2,
                                              space="PSUM"))

        idf = const.tile([128, 128], F32)
        make_identity(nc, idf)
        idb = const.tile([128, 128], BF16)
        make_identity(nc, idb)
        bq_sb = const.tile([128, NPAIRS], F32)
        bk_sb = const.tile([128, NPAIRS], F32)
        bv_sb = const.tile([128, NPAIRS], F32)
        zb = const.tile([128, 512], BF16)
        nc.vector.memset(zb, 0.0)
        zf = const.tile([128, 512], F32)
        nc.vector.memset(zf, 0.0)

        def emit_bias_dmas():
            # first on scalar — the q-nt0 sub3 bias-add is on the
            # critical path to the first score block; partition-
            # restructuring DMAs must stay on sync/scalar queues
            nc.scalar.dma_start(out=bq_sb,
                                in_=bq.rearrange("(p r) -> r p", r=128))
            nc.scalar.dma_start(out=bk_sb,
                                in_=bk.rearrange("(p r) -> r p", r=128))
            nc.scalar.dma_start(out=bv_sb,
                                in_=bv.rearrange("(p r) -> r p", r=128))

        # x^T resident in bf16 (host pre-casts; DMA direct).  Emitted via
        # emit_x_dmas() after pair-0's weight DMAs so projections can
        # start as soon as the first column chunks land.
        xT_sb = persist.tile([128, 4, DC, 512], BF16)

        def emit_x_dmas():
            # one kickoff per nt chunk (1MB, 128 contiguous 8KB
            # descriptors), alternating sync/scalar: nt0 lands ~11us,
            # nt1 by the time k-nt1 is needed (~exp kt4)
            qeng2 = [nc.sync, nc.scalar]
            for nt in range(4):
                qeng2[nt % 2].dma_start(
                    out=xT_sb[:, nt].rearrange("p a b -> p (a b)"),
                    in_=xT[nt])

        wo_sb = persist.tile([128, NPAIRS, D], BF16)

        def emit_wo_dma():
            # 1MB, 512 descriptors — needed only when the output
            # projection starts (pair 3), so it must NOT clog the gpsimd
            # queue ahead of wv (emitted after pair0's setup)
            nc.gpsimd.dma_start(out=wo_sb,
                                in_=wo.rearrange("(p r) n -> r p n", r=128))

        attnT = [persist.tile([128, S], BF16, name=f"attnT{p}",
                              tag=f"attnT{p}") for p in range(NPAIRS)]

        # warmup to hold PE activity while the first DMAs land.  N=128
        # keeps the queue fine-grained so real matmuls start the moment
        # their DMA deps resolve (~9us); sized to end right about then —
        # the real projection stream then carries HAM past its warm
        # threshold.
        warm_ps = scps.tile([128, 1024], F32, name="warm_ps", tag="sc")
        for _ in range(30):
            nc.tensor.matmul(warm_ps[:, 0:128], zb[:, 0:128],
                             zb[:, 0:128],
                             start=True, stop=True, skip_group_check=True)

        pair_tiles = {}
        wre = {"q": wq, "k": wk, "v": wv}
        bias_sb = {"q": bq_sb, "k": bk_sb, "v": bv_sb}

        def gen_pair_work(p, prime=False):
            """Closures producing Q^T (f32r), zero-padded per-head K^T
            (f32r), and V (f32, k-major with ones cols) for pair p.
            Emitted as PE filler during pair p-1's attention.

            For prime (pair 0): returns (inline_ops, filler_ops) where
            inline_ops = setup + q-nt0 + k-nt0 + done (the minimum for
            the first score block) and filler_ops carries the rest in
            dependency order: remaining K chunks first (scores consume
            them at 4 k-tiles per nt chunk), then q-nt1 (needed at qc1),
            then all V work (needed when qc0's AV runs during qc1),
            then q-nt2/q-nt3."""
            ops = []
            st = {}

            def setup():
                weng = {"q": nc.sync, "k": nc.scalar, "v": nc.gpsimd}
                for nm in ("q", "k", "v"):
                    t = wpool.tile([128, DC, 128], BF16, name=f"w{nm}",
                                   tag=f"w{nm}")
                    if not (prime and nm == "v"):
                        weng[nm].dma_start(
                            out=t.rearrange("p a b -> p (a b)"),
                            in_=wre[nm][p])
                    st[nm] = t

                st["qt"] = qkv.tile([128, S], F32R, name="qt_sb", tag="qt")
                st["kt0"] = qkv.tile([128, S], F32R, name="kt0_sb",
                                     tag="kt0")
                st["kt1"] = qkv.tile([128, S], F32R, name="kt1_sb",
                                     tag="kt1")
                st["vt"] = qkv.tile([128, S], BF16, name="vt_sb", tag="vt",
                                    bufs=1)
                st["vsb"] = vpool.tile([128, NKT, 130], BF16, name="v_sb",
                                       tag="vsb")
                nc.vector.memset(st["vsb"][:, :, 64:65], 1.0)
                nc.vector.memset(st["vsb"][:, :, 129:130], 1.0)
                zrep = bass.AP(tensor=zf.tensor, offset=zf.offset,
                               ap=[zf.ap[0], [0, 4], [1, 512]])
                nc.vector.tensor_copy(
                    st["kt0"].rearrange("p (a b) -> p a b", b=512), zrep)
                nc.vector.tensor_copy(
                    st["kt1"].rearrange("p (a b) -> p a b", b=512), zrep)
            ops.append(setup)

            def wv_kick():
                # prime only: wv rides gpsimd AFTER the nt0 x chunks
                # (needed ~20us later than them)
                nc.gpsimd.dma_start(
                    out=st["v"].rearrange("p a b -> p (a b)"),
                    in_=wre["v"][p])

            pstate = {}

            def proj(nm, nt, sub):
                def go():
                    if sub == 0:
                        pstate["pp"] = fips.tile([128, 512], F32, name="pp",
                                                 tag="pp")
                    pp = pstate["pp"]
                    for dc in range(2 * sub, 2 * sub + 2):
                        nc.tensor.matmul(
                            pp, st[nm][:, dc, :],
                            xT_sb[:, nt, dc, :],
                            start=(dc == 0), stop=(dc == DC - 1))
                    if sub == 3:
                        csl = slice(nt * 512, (nt + 1) * 512)
                        b = bias_sb[nm][:, p:p + 1]
                        if nm == "q":
                            nc.vector.tensor_scalar_add(st["qt"][:, csl],
                                                        pp, b)
                        elif nm == "v":
                            nc.vector.tensor_scalar_add(st["vt"][:, csl],
                                                        pp, b)
                        else:
                            nc.vector.tensor_scalar_add(
                                st["kt0"][0:64, csl], pp[0:64, :],
                                b[0:64, :])
                            nc.vector.tensor_scalar_add(
                                st["kt1"][64:128, csl], pp[64:128, :],
                                b[64:128, :])
                return go

            def vtr(g4):
                # PE transpose (bf16 vt -> 1 cyc/row, half the old f32
                # cost; DMA XBAR transposes measured 1.25us each on the
                # Sync engine and choke the v_sb dependency chain)
                def go():
                    tr = fips.tile([128, 512], BF16, name="tr", tag="pp")
                    for j in range(4):
                        kb = g4 * 4 + j
                        nc.tensor.matmul(
                            tr[:, j * 128:(j + 1) * 128],
                            st["vt"][:, kb * 128:(kb + 1) * 128], idb,
                            is_transpose=True, start=True, stop=True)
                    src0 = bass.AP(tensor=tr.tensor, offset=tr.offset,
                                   ap=[tr.ap[0], [128, 4], [1, 64]])
                    src1 = bass.AP(tensor=tr.tensor,
                                   offset=tr.offset + 64,
                                   ap=[tr.ap[0], [128, 4], [1, 64]])
                    vs = st["vsb"]
                    nc.vector.tensor_copy(
                        vs[:, g4 * 4:g4 * 4 + 4, 0:64], src0)
                    nc.vector.tensor_copy(
                        vs[:, g4 * 4:g4 * 4 + 4, 65:129], src1)
                return go

            def done():
                pair_tiles[p] = st

            if prime:
                # inline: the minimum to open the first score block
                for nm in ("q", "k"):
                    for sub in range(4):
                        ops.append(proj(nm, 0, sub))
                ops.append(done)
                # filler order tuned against consumption pops: remaining
                # K chunks (scores eat 4 k-tiles per nt), v-nt0+vtr0
                # (qc0's AV starts at qc1-kt0), q-nt1 (qc1's scores),
                # the rest of V (AV eats vtr g4 at qc1-kt 4*g4), then
                # q-nt2/3 (qc2/qc3)
                rest = []
                for nt in range(1, 4):
                    for sub in range(4):
                        rest.append(proj("k", nt, sub))
                for sub in range(4):
                    rest.append(proj("v", 0, sub))
                rest.append(vtr(0))
                for sub in range(4):
                    rest.append(proj("q", 1, sub))
                for nt in range(1, 4):
                    for sub in range(4):
                        rest.append(proj("v", nt, sub))
                    rest.append(vtr(nt))
                for nt in (2, 3):
                    for sub in range(4):
                        rest.append(proj("q", nt, sub))
                return ops, rest, wv_kick

            # non-prime: Q/K first, then done (so the handles exist well
            # before the next pair's emission and the boundary drain is a
            # no-op), then V work — consumed only by the pair's own AV
            # slots, which lag a full q-chunk behind.
            for nm in ("q", "k"):
                for nt in range(4):
                    for sub in range(4):
                        ops.append(proj(nm, nt, sub))
            ops.append(done)
            for nt in range(4):
                for sub in range(4):
                    ops.append(proj("v", nt, sub))
                ops.append(vtr(nt))
            return ops

        filler = deque()
        avq = deque()

        def gen_outproj_qci(qci):
            """4 half-closures (2 matmuls each) computing the output
            projection for q rows qci*128..+128.  Requires attnT[p][:,
            those cols] for all pairs.  Split so a single filler pop
            never exceeds the exp cadence."""
            ops = []
            ostate = {}
            for nt in range(2):
                def goA(nt=nt):
                    op = fips.tile([128, 512], F32, name="op", tag="pp")
                    ostate[nt] = op
                    for pi in (0, 1):
                        nc.tensor.matmul(
                            op,
                            attnT[pi][:, qci * 128:(qci + 1) * 128],
                            wo_sb[:, pi, nt * 512:(nt + 1) * 512],
                            start=(pi == 0), stop=False,
                            skip_group_check=True)

                def goB(nt=nt):
                    op = ostate[nt]
                    for pi in (2, 3):
                        nc.tensor.matmul(
                            op,
                            attnT[pi][:, qci * 128:(qci + 1) * 128],
                            wo_sb[:, pi, nt * 512:(nt + 1) * 512],
                            start=False, stop=(pi == NPAIRS - 1),
                            skip_group_check=True)
                    o = osb.tile([128, 512], BF16, name="o", tag="o")
                    # the last q-chunk's evictions + out-DMAs run in the
                    # drain when the ACT engine is done with exps — use
                    # it (and its queue) to unload DVE/gpsimd tail
                    # serialization
                    if qci >= 12:
                        nc.scalar.copy(o, op)
                        nc.scalar.dma_start(
                            out=out[qci * 128:(qci + 1) * 128,
                                    nt * 512:(nt + 1) * 512], in_=o)
                    else:
                        nc.vector.tensor_copy(o, op)
                        nc.gpsimd.dma_start(
                            out=out[qci * 128:(qci + 1) * 128,
                                    nt * 512:(nt + 1) * 512], in_=o)
                ops.append(goA)
                ops.append(goB)
            return ops

        def gen_av_slot_factory(p, qc, es_list, v_sb):
            """Returns slot(qt4, sub) closure factory for q-chunk qc of
            pair p.  attn[q,dh] with es as the STATIONARY operand: per
            (qt4, sub) slot, 8 small matmuls (4 k-tiles x 2 heads, 65
            moving rows each) accumulate [128,130] =
            [attn_h0|sum_h0|attn_h1|sum_h1].  The sumexp lands as a
            per-partition column -> normalize is a plain tensor_scalar
            multiply straight from PSUM, and a DMA XBAR transpose drops
            the bf16 result into the attnT layout."""
            q0 = qc * 512
            state = {}

            def slot(qt4, sub):
                def go():
                    if sub == 0:
                        avfull = avps.tile([128, 512], F32, name="av",
                                           tag=f"av{qt4 % 2}")
                        state[qt4] = avfull[:, 0:130]
                    av = state[qt4]
                    for kt in range(4 * sub, 4 * sub + 4):
                        for hh in range(2):
                            esl = es_list[kt][:, hh * 512 + qt4 * 128:
                                              hh * 512 + (qt4 + 1) * 128]
                            nc.tensor.matmul(
                                av[:, hh * 65:(hh + 1) * 65], esl,
                                v_sb[:, kt, hh * 65:(hh + 1) * 65],
                                start=(kt == 0 and hh == 0),
                                stop=(kt == NKT - 1),
                                skip_group_check=True)
                    if sub == 3:
                        rec = nrm.tile([128, 2], F32, name="rec",
                                       tag="rec", bufs=3)
                        av_sums = bass.AP(tensor=av.tensor,
                                          offset=av.offset + 64,
                                          ap=[av.ap[0], [65, 2]])
                        nc.vector.reciprocal(rec, av_sums)
                        att = nrm.tile([128, 128], BF16, name="att",
                                       tag="att", bufs=3)
                        nc.vector.tensor_scalar_mul(
                            att[:, 0:64], av[:, 0:64], rec[:, 0:1])
                        nc.vector.tensor_scalar_mul(
                            att[:, 64:128], av[:, 65:129], rec[:, 1:2])
                        # PE transpose (bf16: 1 cyc/row, ~58ns; the DMA
                        # XBAR path costs 1.25us on the Sync engine and
                        # serializes the drain)
                        trn = fips.tile([128, 128], BF16, name="trn",
                                        tag="pp")
                        nc.tensor.matmul(trn, att, idb,
                                         is_transpose=True,
                                         start=True, stop=True)
                        nc.vector.tensor_copy(
                            attnT[p][:,
                                     q0 + qt4 * 128:q0 + (qt4 + 1) * 128],
                            trn)
                        if p == NPAIRS - 1:
                            filler.extend(gen_outproj_qci(qc * 4 + qt4))
                return go
            return slot

        def attention(p):
            while p not in pair_tiles:
                filler.popleft()()
            st = pair_tiles[p]
            qt_sb, kt0, kt1 = st["qt"], st["kt0"], st["kt1"]
            v_sb = st["vsb"]
            if p < NPAIRS - 1:
                filler.extend(gen_pair_work(p + 1))
            for qc in range(NQC):
                last = (p == NPAIRS - 1 and qc == NQC - 1)
                q0 = qc * 512
                es_q = []
                mkslot = gen_av_slot_factory(p, qc, es_q, v_sb)
                for kt in range(NKT):
                    sc = scps.tile([128, 1024], F32, name="sc", tag="sc")
                    nc.tensor.matmul(
                        sc[:, 0:512],
                        kt0[:, kt * 128:(kt + 1) * 128],
                        qt_sb[:, q0:q0 + 512], start=True, stop=True)
                    nc.tensor.matmul(
                        sc[:, 512:1024],
                        kt1[:, kt * 128:(kt + 1) * 128],
                        qt_sb[:, q0:q0 + 512], start=True, stop=True)
                    es = espool.tile([128, 1024], BF16, name="es",
                                     tag="es")
                    nc.scalar.activation(es, sc, EXP, scale=0.125)
                    es_q.append(es)
                    # adaptive: drain the filler backlog (pair-boundary
                    # bursts starve the exp stream otherwise)
                    npop = 1 + (1 if not avq else 0) \
                        + (1 if len(filler) > 28 else 0)
                    for _ in range(npop):
                        if filler:
                            filler.popleft()()
                    for _ in range(2 if last else 1):
                        if avq:
                            avq.popleft()()
                    if last and kt % 4 == 3:
                        g = kt // 4
                        avq.append(mkslot(0, g))
                        avq.append(mkslot(1, g))
                        if kt == NKT - 1:
                            for t in (2, 3):
                                for g2 in range(4):
                                    avq.append(mkslot(t, g2))
                if not last:
                    for qt4 in range(4):
                        for sub in range(4):
                            avq.append(mkslot(qt4, sub))

        # prime pair 0 (runs while x DMAs land; warmup holds the clock).
        # Pair-0 weight DMAs go out before the bulk x load so the first
        # projection matmuls are not starved.
        prime_ops, prime_rest, wv_kick = gen_pair_work(0, prime=True)
        emit_bias_dmas()
        prime_ops[0]()
        emit_x_dmas()
        wv_kick()
        emit_wo_dma()
        for op in prime_ops[1:]:
            op()
        filler.extend(prime_rest)

        for p in range(NPAIRS):
            attention(p)
        while avq:
            avq.popleft()()
            if filler:
                filler.popleft()()
        while filler:
            filler.popleft()()

    nc.compile()
    return nc


def _get_compiled():
    global _COMPILED
    if _COMPILED is None:
        _COMPILED = _build()
    return _COMPILED


def make_in_maps(**inputs):
    import ml_dtypes
    bf = ml_dtypes.bfloat16
    x = np.asarray(inputs["inputs"], np.float32)
    # [nt, p, dc, 512]: per-partition-contiguous nt chunks (see kernel)
    xTb = [np.ascontiguousarray(
        x[b].T.reshape(DC, 128, 4, 512).transpose(2, 1, 0, 3)
        .reshape(4, 128, DC * 512)).astype(bf) for b in range(B)]
    gslice = {}
    for nm in ("Wq", "Wk", "Wv", "Wo", "bq", "bk", "bv"):
        a = np.asarray(inputs[nm], np.float32)
        for g in range(2):
            sl = slice(g * GCOLS, (g + 1) * GCOLS)
            if nm == "Wo":
                gslice[(nm, g)] = np.ascontiguousarray(a[sl, :]).astype(bf)
            elif nm.startswith("W"):
                w = a[:, sl].reshape(8, 128, 4, 128).transpose(1, 2, 0, 3)
                gslice[(nm, g)] = np.ascontiguousarray(
                    w.transpose(1, 0, 2, 3).reshape(4, 128, 1024)).astype(bf)
            else:
                gslice[(nm, g)] = np.ascontiguousarray(a[sl])
    in_maps = []
    for c in range(NCORES):
        g, b = c // B, c % B
        in_maps.append({
            "xT": xTb[b],
            "wq": gslice[("Wq", g)], "wk": gslice[("Wk", g)],
            "wv": gslice[("Wv", g)], "wo": gslice[("Wo", g)],
            "bq": gslice[("bq", g)], "bk": gslice[("bk", g)],
            "bv": gslice[("bv", g)],
        })
    return in_maps


def combine(results, bo):
    out = np.empty((B, S, D), np.float32)
    bo = np.asarray(bo, np.float32)
    for b in range(B):
        out[b] = (np.asarray(results[b]["out"], np.float32)
                  + np.asarray(results[B + b]["out"], np.float32) + bo)
    return out


def kernel(**inputs):
    from concourse import bass_utils
    nc = _get_compiled()
    in_maps = make_in_maps(**inputs)
    res = bass_utils.run_bass_kernel_spmd(
        nc, in_maps, core_ids=list(range(NCORES)))
    return combine(res.results, inputs["bo"])
